# revision 5
# baseline (speedup 1.0000x reference)
"""Domain-specific BatchNorm (training mode) Trainium2 Bass kernel, v5.

Feature-sharded, collective-free: each of 8 cores owns 128 of 1024
features for ALL rows; host stable-sorts rows by domain and ships x
transposed ([128 feat, S cols], fp16) with each domain's column range
padded to a 32-col boundary (pad cols zeroed, excluded from stats).

Stats (per domain, arrival-aware engine split):
  - early domains (0-2): all-DVE bn_stats (512-col blocks) + bn_aggr;
    the <512-col remainder goes to ACT (idle early).
  - late domains (3-7): alternating 3/2 bn blocks on DVE, remainder on
    ACT as Identity(scale=1/n)+accum and Square(scale=sqrt(1/n))+accum
    (pre-scaled partials, no divides later).
Merge + scale/offset math runs on GpSimd for early batches (DVE/ACT are
busy), on DVE for the last batch (short tail); sqrt on ACT, reciprocal
on DVE.  Apply is one 2048-wide tensor_scalar (x*scale+off, 4x DVE
mode) per domain, spread across DVE/ACT/GpSimd.  Loads: 2 chunks per
domain on the sync queue; stores follow each apply (sync queue early,
scalar queue late).
"""

import math
import os
import sys

import numpy as np

for _p in ("/opt/trn_rl_repo", "/root/.axon_site/_ro/trn_rl_repo"):
    if os.path.isdir(_p) and _p not in sys.path:
        sys.path.insert(0, _p)

import concourse.bass as bass
import concourse.tile as tile
from concourse import mybir
from concourse.bass_utils import run_bass_kernel_spmd

N_CORES = 8
N, F, D = 16384, 1024, 8
FC = F // N_CORES  # features per core (128)
EPS = 1e-5
ALIGN = 32
BNB = 512           # bn_stats block width (hw max free size)
NBMAX = 8           # max bn blocks per domain we allocate for
APPLY_CHUNK = 2048  # DVE 4x tensor_scalar op width

F32 = mybir.dt.float32
F16 = mybir.dt.float16
AF = mybir.ActivationFunctionType
OP = mybir.AluOpType

N_EARLY = 3  # domains with all-DVE stats
# apply engine per domain index: v=vector, a=scalar, g=gpsimd
APPLY_ENG = {0: "g", 1: "v", 2: "g", 3: "a", 4: "v", 5: "a", 6: "v", 7: "v"}


def _split_multiwait_instructions(nc):
    """Walrus codegen encodes at most ONE sync wait per engine instruction.
    Tile may attach several; hoist all but the last into standalone
    InstEventSemaphore instructions on the same engine, placed before."""
    n = 0
    for fn in nc.m.functions:
        for block in fn.blocks:
            out = []
            for inst in block.instructions:
                si = inst.sync_info
                waits = list(si.on_wait) if si is not None else []
                if len(waits) > 1:
                    for w in waits[:-1]:
                        ev = mybir.InstEventSemaphore(
                            name=f"{inst.name}-ws{n}", ins=[], outs=[]
                        )
                        ev.engine = inst.engine
                        ev.sync_info = mybir.SyncInfo(on_wait=[w], on_update=[])
                        out.append(ev)
                        n += 1
                    inst.sync_info = mybir.SyncInfo(
                        on_wait=[waits[-1]], on_update=list(si.on_update)
                    )
                out.append(inst)
            block.instructions = out
    return n


def _plan(counts):
    """Per-domain layout: padded starts, bn-block counts, engine split."""
    rngs = []
    a = 0
    for d, c in enumerate(counts):
        c = int(c)
        nb = 0
        if c > 1:
            if d < N_EARLY:
                nb = c // BNB                      # remainder < 512 -> ACT
            else:
                nb = max(c // BNB - (1 if d % 2 == 0 else 2), 0)
            nb = min(nb, NBMAX)
        rngs.append({"a": a, "w": c, "nb": nb, "wb": c - nb * BNB, "d": d})
        a += (c + ALIGN - 1) // ALIGN * ALIGN
    return rngs, max(a, ALIGN)


def _build_kernel(counts):
    rngs, S_pad = _plan(counts)
    nc = bass.Bass("TRN2", target_bir_lowering=False, debug=False,
                   num_devices=N_CORES)
    x_d = nc.dram_tensor("x", [FC, S_pad], F16, kind="ExternalInput")
    aux_d = nc.dram_tensor("aux", [FC, 3 * D + 1], F32, kind="ExternalInput")
    o_d = nc.dram_tensor("out", [FC, S_pad], F16, kind="ExternalOutput")

    with tile.TileContext(nc) as tc:
        _body(tc, counts, rngs, S_pad, x_d, aux_d, o_d)
    return nc


def _body(tc, counts, rngs, S_pad, x_d, aux_d, o_d):
    nc = tc.nc
    from contextlib import ExitStack

    live = [r for r in rngs if r["w"] > 0]
    batches = [[r for r in live if r["d"] < 3],
               [r for r in live if 3 <= r["d"] < 6],
               [r for r in live if r["d"] >= 6]]
    batches = [b for b in batches if b]

    with ExitStack() as ctx:
        big = ctx.enter_context(tc.tile_pool(name="big", bufs=1))
        small = ctx.enter_context(tc.tile_pool(name="small", bufs=1))

        xt = big.tile([FC, S_pad], F16)
        ot = big.tile([FC, S_pad], F16)
        wb_max = max((r["wb"] for r in live), default=1)
        scr_a = big.tile([FC, max(wb_max, 1)], F16)   # ACT activation out

        aux = small.tile([FC, 3 * D + 1], F32)
        ga = aux[:, 0:D]
        be = aux[:, D:2 * D]
        eps_t = aux[:, 2 * D:2 * D + 1]
        fa = aux[:, 2 * D + 1:3 * D + 1]  # n_dve/n per domain

        nc.gpsimd.dma_start(aux[:, :], aux_d[:, :])

        bnout = small.tile([FC, D * NBMAX * 6], F32)
        aggr = small.tile([FC, 2 * D], F32)
        meanb = small.tile([FC, D], F32)
        msqb = small.tile([FC, D], F32)
        nc.gpsimd.memset(aggr[:, :], 0.0)
        nc.gpsimd.memset(meanb[:, :], 0.0)
        nc.gpsimd.memset(msqb[:, :], 0.0)

        m2a = small.tile([FC, D], F32)
        ex2 = small.tile([FC, D], F32)
        mean = small.tile([FC, D], F32)
        mt2 = small.tile([FC, D], F32)
        var = small.tile([FC, D], F32)
        sd = small.tile([FC, D], F32)
        inv = small.tile([FC, D], F32)
        scale = small.tile([FC, D], F32)
        ms = small.tile([FC, D], F32)
        off = small.tile([FC, D], F32)

        # ---- loads: two chunks per domain, sync queue ------------------
        pads = []
        for i, r in enumerate(rngs):
            a, w = r["a"], r["w"]
            end = rngs[i + 1]["a"] if i + 1 < len(rngs) else S_pad
            if w > 0:
                h = (w // 2 + BNB - 1) // BNB * BNB  # chunk split on a bn edge
                h = min(h, w)
                nc.sync.dma_start(xt[:, a:a + h], x_d[:, a:a + h])
                if h < w:
                    nc.sync.dma_start(xt[:, a + h:a + w], x_d[:, a + h:a + w])
            if a + w < end:
                pads.append((a + w, end))
        for (p0, p1) in pads:
            nc.gpsimd.memset(xt[:, p0:p1], 0.0)

        # ---- stats per domain ----------------------------------------
        def stats(r):
            a, w, nb, wb, d = r["a"], r["w"], r["nb"], r["wb"], r["d"]
            if w <= 1:
                return
            rc = 1.0 / w
            for k in range(nb):
                nc.vector.bn_stats(
                    bnout[:, (d * NBMAX + k) * 6:(d * NBMAX + k + 1) * 6],
                    xt[:, a + k * BNB:a + (k + 1) * BNB],
                )
            if nb > 0:
                nc.vector.bn_aggr(
                    aggr[:, 2 * d:2 * d + 2],
                    bnout[:, d * NBMAX * 6:(d * NBMAX) * 6 + nb * 6],
                )
            if wb > 0:
                xin = xt[:, a + nb * BNB:a + w]
                nc.scalar.activation(
                    scr_a[:, 0:wb], xin, AF.Identity, scale=rc,
                    accum_out=meanb[:, d:d + 1],
                )
                nc.scalar.activation(
                    scr_a[:, 0:wb], xin, AF.Square, scale=math.sqrt(rc),
                    accum_out=msqb[:, d:d + 1],
                )

        # ---- merge + scale/offset math for a batch --------------------
        def phase_b(batch, e):
            lo = min(r["d"] for r in batch)
            hi = max(r["d"] for r in batch) + 1
            cs = slice(lo, hi)
            ma = aggr[:, 2 * lo:2 * hi:2]
            va = aggr[:, 2 * lo + 1:2 * hi:2]
            e.tensor_tensor(m2a[:, cs], ma, ma, OP.mult)
            e.tensor_tensor(ex2[:, cs], va, m2a[:, cs], OP.add)
            e.tensor_tensor(ex2[:, cs], ex2[:, cs], fa[:, cs], OP.mult)
            e.tensor_tensor(ex2[:, cs], ex2[:, cs], msqb[:, cs], OP.add)
            e.tensor_tensor(mean[:, cs], ma, fa[:, cs], OP.mult)
            e.tensor_tensor(mean[:, cs], mean[:, cs], meanb[:, cs], OP.add)
            e.tensor_tensor(mt2[:, cs], mean[:, cs], mean[:, cs], OP.mult)
            e.tensor_tensor(var[:, cs], ex2[:, cs], mt2[:, cs], OP.subtract)
            nc.scalar.activation(sd[:, cs], var[:, cs], AF.Sqrt,
                                 bias=eps_t[:, 0:1])
            nc.vector.reciprocal(inv[:, cs], sd[:, cs])
            e.tensor_tensor(scale[:, cs], inv[:, cs], ga[:, cs], OP.mult)
            e.tensor_tensor(ms[:, cs], mean[:, cs], scale[:, cs], OP.mult)
            e.tensor_tensor(off[:, cs], be[:, cs], ms[:, cs], OP.subtract)
            for r in batch:
                if r["w"] == 1:  # passthrough: out = x
                    nc.vector.memset(scale[:, r["d"]:r["d"] + 1], 1.0)
                    nc.vector.memset(off[:, r["d"]:r["d"] + 1], 0.0)

        # ---- apply + store --------------------------------------------
        def apply_store(r, idx):
            a, w, d = r["a"], r["w"], r["d"]
            we = w + (w % 2)
            sc_col = scale[:, d:d + 1]
            of_col = off[:, d:d + 1]
            eng = {"v": nc.vector, "a": nc.scalar,
                   "g": nc.gpsimd}[APPLY_ENG.get(d, "v")]
            if eng is nc.scalar:
                nc.scalar.activation(ot[:, a:a + we], xt[:, a:a + we],
                                     AF.Identity, bias=of_col, scale=sc_col)
            else:
                p = a
                while p < a + we:
                    q = min(p + APPLY_CHUNK, a + we)
                    eng.tensor_scalar(
                        ot[:, p:q], xt[:, p:q], sc_col, of_col,
                        OP.mult, OP.add,
                    )
                    p = q
            qeng = nc.sync if idx < 5 else nc.scalar
            qeng.dma_start(o_d[:, a:a + we], ot[:, a:a + we])

        # ---- emission --------------------------------------------------
        for bi, batch in enumerate(batches):
            for r in batch:
                stats(r)
            pb_eng = nc.gpsimd if bi < len(batches) - 1 else nc.vector
            phase_b(batch, pb_eng)
            for r in batch:
                apply_store(r, rngs.index(r))


_NC_CACHE = {}


def _get_nc(counts):
    key = tuple(int(c) for c in counts)
    if key not in _NC_CACHE:
        nc = _build_kernel(key)
        _split_multiwait_instructions(nc)
        _NC_CACHE[key] = nc
    return _NC_CACHE[key]


def _prep(inputs):
    x = np.asarray(inputs["x"])
    y = np.asarray(inputs["y"]).astype(np.int64)
    gamma = np.asarray(inputs["gamma"], dtype=np.float32)
    beta = np.asarray(inputs["beta"], dtype=np.float32)
    n, f = x.shape
    d = gamma.shape[0]

    counts = np.bincount(y, minlength=d).astype(np.int64)
    perm = np.argsort(y, kind="stable")
    xs = x[perm].astype(np.float16)

    rngs, S_pad = _plan(counts)
    xs_p = np.zeros((S_pad, f), dtype=np.float16)
    pos = np.empty(n, dtype=np.int64)
    o = 0
    for r in rngs:
        c = r["w"]
        xs_p[r["a"]:r["a"] + c] = xs[o:o + c]
        pos[o:o + c] = np.arange(r["a"], r["a"] + c)
        o += c

    fa = np.array([(r["nb"] * BNB / r["w"]) if r["w"] > 0 else 0.0
                   for r in rngs], dtype=np.float32)
    return x, counts, perm, pos, xs_p, gamma, beta, fa, S_pad


def _run(inputs, trace=False, **kw):
    x, counts, perm, pos, xs_p, gamma, beta, fa, S_pad = _prep(inputs)
    n, f = x.shape
    d = gamma.shape[0]

    nc = _get_nc(counts)
    in_maps = []
    for c in range(N_CORES):
        sl = slice(c * FC, (c + 1) * FC)
        aux = np.empty((FC, 3 * D + 1), dtype=np.float32)
        aux[:, 0:D] = gamma[:, sl].T
        aux[:, D:2 * D] = beta[:, sl].T
        aux[:, 2 * D] = EPS
        aux[:, 2 * D + 1:] = fa[None, :]
        in_maps.append(
            {
                "x": np.ascontiguousarray(xs_p[:, sl].T),
                "aux": aux,
            }
        )
    res = run_bass_kernel_spmd(
        nc, in_maps, core_ids=list(range(N_CORES)), trace=trace, **kw
    )
    out_s = np.empty((n, f), dtype=np.float32)
    for c in range(N_CORES):
        sl = slice(c * FC, (c + 1) * FC)
        out_s[:, sl] = res.results[c]["out"].T[pos]
    out = np.empty_like(out_s)
    out[perm] = out_s
    return out, res


def kernel(**inputs) -> np.ndarray:
    out, _ = _run(inputs, trace=False)
    return out


# revision 18
# speedup vs baseline: 1.0509x; 1.0509x over previous
"""Domain-specific BatchNorm (training mode) Trainium2 Bass kernel, v5.

Feature-sharded, collective-free: each of 8 cores owns 128 of 1024
features for ALL rows; host stable-sorts rows by domain and ships x
transposed ([128 feat, S cols], fp16) with each domain's column range
padded to a 32-col boundary (pad cols zeroed, excluded from stats).

Stats (per domain, arrival-aware engine split):
  - early domains (0-2): all-DVE bn_stats (512-col blocks) + bn_aggr;
    the <512-col remainder goes to ACT (idle early).
  - late domains (3-7): alternating 3/2 bn blocks on DVE, remainder on
    ACT as Identity(scale=1/n)+accum and Square(scale=sqrt(1/n))+accum
    (pre-scaled partials, no divides later).
Merge + scale/offset math runs on GpSimd for early batches (DVE/ACT are
busy), on DVE for the last batch (short tail); sqrt on ACT, reciprocal
on DVE.  Apply is one 2048-wide tensor_scalar (x*scale+off, 4x DVE
mode) per domain, spread across DVE/ACT/GpSimd.  Loads: 2 chunks per
domain on the sync queue; stores follow each apply (sync queue early,
scalar queue late).
"""

import math
import os
import sys

import numpy as np

for _p in ("/opt/trn_rl_repo", "/root/.axon_site/_ro/trn_rl_repo"):
    if os.path.isdir(_p) and _p not in sys.path:
        sys.path.insert(0, _p)

import concourse.bass as bass
import concourse.tile as tile
from concourse import mybir
from concourse.bass_utils import run_bass_kernel_spmd

N_CORES = 8
N, F, D = 16384, 1024, 8
FC = F // N_CORES  # features per core (128)
EPS = 1e-5
ALIGN = 32
BNB = 512           # bn_stats block width (hw max free size)
NBMAX = 8           # max bn blocks per domain we allocate for
APPLY_CHUNK = 4096  # one tensor_scalar op per domain

F32 = mybir.dt.float32
F16 = mybir.dt.float16
AF = mybir.ActivationFunctionType
OP = mybir.AluOpType

N_EARLY = 3  # domains with all-DVE stats
# apply engine per domain index: v=vector, a=scalar, g=gpsimd
APPLY_ENG = {0: "g", 1: "v", 2: "g", 3: "g", 4: "g", 5: "g", 6: "v", 7: "v"}


def _split_multiwait_instructions(nc):
    """Walrus codegen encodes at most ONE sync wait per engine instruction.
    Tile may attach several; hoist all but the last into standalone
    InstEventSemaphore instructions on the same engine, placed before."""
    n = 0
    for fn in nc.m.functions:
        for block in fn.blocks:
            out = []
            for inst in block.instructions:
                si = inst.sync_info
                waits = list(si.on_wait) if si is not None else []
                if len(waits) > 1:
                    for w in waits[:-1]:
                        ev = mybir.InstEventSemaphore(
                            name=f"{inst.name}-ws{n}", ins=[], outs=[]
                        )
                        ev.engine = inst.engine
                        ev.sync_info = mybir.SyncInfo(on_wait=[w], on_update=[])
                        out.append(ev)
                        n += 1
                    inst.sync_info = mybir.SyncInfo(
                        on_wait=[waits[-1]], on_update=list(si.on_update)
                    )
                out.append(inst)
            block.instructions = out
    return n


def _plan(counts):
    """Per-domain layout: padded starts, bn-block counts, engine split."""
    rngs = []
    a = 0
    for d, c in enumerate(counts):
        c = int(c)
        nb = 0
        if c > 1:
            # bn_aggr assumes equal-count groups: blocks must be exactly 512
            if d < N_EARLY:
                nb = c // BNB  # remainder (<512) via DVE sum ops
            else:
                nb = max(min(c // BNB, 1 if d == 4 else 2), 0)
            nb = min(nb, NBMAX)
        cov = nb * BNB  # columns covered by bn blocks
        wb = c - cov
        rngs.append({"a": a, "w": c, "nb": nb, "cov": cov, "wb": wb, "d": d})
        a += (c + ALIGN - 1) // ALIGN * ALIGN
    return rngs, max(a, ALIGN)


def _build_kernel(counts):
    rngs, S_pad = _plan(counts)
    nc = bass.Bass("TRN2", target_bir_lowering=False, debug=False,
                   num_devices=N_CORES)
    x_d = nc.dram_tensor("x", [FC, S_pad], F16, kind="ExternalInput")
    aux_d = nc.dram_tensor("aux", [FC, 3 * D + 1], F32, kind="ExternalInput")
    o_d = nc.dram_tensor("out", [FC, S_pad], F16, kind="ExternalOutput")

    with tile.TileContext(nc) as tc:
        _body(tc, counts, rngs, S_pad, x_d, aux_d, o_d)
    return nc


def _body(tc, counts, rngs, S_pad, x_d, aux_d, o_d):
    nc = tc.nc
    from contextlib import ExitStack

    live = [r for r in rngs if r["w"] > 0]
    batches = [[r for r in live if r["d"] < 3],
               [r for r in live if 3 <= r["d"] < 6],
               [r for r in live if r["d"] >= 6]]
    batches = [b for b in batches if b]

    with ExitStack() as ctx:
        big = ctx.enter_context(tc.tile_pool(name="big", bufs=1))
        small = ctx.enter_context(tc.tile_pool(name="small", bufs=1))

        xt = big.tile([FC, S_pad], F16)
        ot = big.tile([FC, S_pad], F16)
        wb_max = max((r["wb"] for r in live), default=1)
        scr_a = big.tile([FC, max(wb_max, 1)], F16)   # ACT activation out

        aux = small.tile([FC, 3 * D + 1], F32)
        ga = aux[:, 0:D]
        be = aux[:, D:2 * D]
        eps_t = aux[:, 2 * D:2 * D + 1]
        fa = aux[:, 2 * D + 1:3 * D + 1]  # n_dve/n per domain

        nc.gpsimd.dma_start(aux[:, :], aux_d[:, :])

        bnout = small.tile([FC, D * NBMAX * 6], F32)
        aggr = small.tile([FC, 2 * D], F32)
        meanb = small.tile([FC, D], F32)
        msqb = small.tile([FC, D], F32)
        nc.gpsimd.memset(aggr[:, :], 0.0)
        nc.gpsimd.memset(meanb[:, :], 0.0)
        nc.gpsimd.memset(msqb[:, :], 0.0)

        m2a = small.tile([FC, D], F32)
        ex2 = small.tile([FC, D], F32)
        mean = small.tile([FC, D], F32)
        mt2 = small.tile([FC, D], F32)
        var = small.tile([FC, D], F32)
        sd = small.tile([FC, D], F32)
        inv = small.tile([FC, D], F32)
        scale = small.tile([FC, D], F32)
        ms = small.tile([FC, D], F32)
        off = small.tile([FC, D], F32)

        # ---- loads: one chunk per domain, sync queue ------------------
        pads = []
        for i, r in enumerate(rngs):
            a, w = r["a"], r["w"]
            end = rngs[i + 1]["a"] if i + 1 < len(rngs) else S_pad
            if w > 0:
                nc.sync.dma_start(xt[:, a:a + w], x_d[:, a:a + w])
            if a + w < end:
                pads.append((a + w, end))
        for (p0, p1) in pads:
            nc.gpsimd.memset(xt[:, p0:p1], 0.0)

        # ---- stats per domain ----------------------------------------
        def stats(r):
            a, w, nb, wb, d = r["a"], r["w"], r["nb"], r["wb"], r["d"]
            if w <= 1:
                return
            rc = 1.0 / w
            for k in range(nb):
                nc.vector.bn_stats(
                    bnout[:, (d * NBMAX + k) * 6:(d * NBMAX + k + 1) * 6],
                    xt[:, a + k * BNB:a + (k + 1) * BNB],
                )
            if nb > 0:
                nc.vector.bn_aggr(
                    aggr[:, 2 * d:2 * d + 2],
                    bnout[:, d * NBMAX * 6:(d * NBMAX) * 6 + nb * 6],
                )
            if wb > 0:
                xin = xt[:, a + r["cov"]:a + w]
                nc.scalar.activation(
                    scr_a[:, 0:wb], xin, AF.Identity, scale=rc,
                    accum_out=meanb[:, d:d + 1],
                )
                nc.scalar.activation(
                    scr_a[:, 0:wb], xin, AF.Square, scale=math.sqrt(rc),
                    accum_out=msqb[:, d:d + 1],
                )

        # ---- merge + scale/offset math for a batch --------------------
        def phase_b(batch, e):
            lo = min(r["d"] for r in batch)
            hi = max(r["d"] for r in batch) + 1
            cs = slice(lo, hi)
            ma = aggr[:, 2 * lo:2 * hi:2]
            va = aggr[:, 2 * lo + 1:2 * hi:2]
            e.tensor_tensor(m2a[:, cs], ma, ma, OP.mult)
            e.tensor_tensor(ex2[:, cs], va, m2a[:, cs], OP.add)
            e.tensor_tensor(ex2[:, cs], ex2[:, cs], fa[:, cs], OP.mult)
            e.tensor_tensor(ex2[:, cs], ex2[:, cs], msqb[:, cs], OP.add)
            e.tensor_tensor(mean[:, cs], ma, fa[:, cs], OP.mult)
            e.tensor_tensor(mean[:, cs], mean[:, cs], meanb[:, cs], OP.add)
            e.tensor_tensor(mt2[:, cs], mean[:, cs], mean[:, cs], OP.mult)
            e.tensor_tensor(var[:, cs], ex2[:, cs], mt2[:, cs], OP.subtract)
            nc.scalar.activation(sd[:, cs], var[:, cs], AF.Sqrt,
                                 bias=eps_t[:, 0:1])
            nc.vector.reciprocal(inv[:, cs], sd[:, cs])
            e.tensor_tensor(scale[:, cs], inv[:, cs], ga[:, cs], OP.mult)
            e.tensor_tensor(ms[:, cs], mean[:, cs], scale[:, cs], OP.mult)
            e.tensor_tensor(off[:, cs], be[:, cs], ms[:, cs], OP.subtract)
            for r in batch:
                if r["w"] == 1:  # passthrough: out = x
                    nc.vector.memset(scale[:, r["d"]:r["d"] + 1], 1.0)
                    nc.vector.memset(off[:, r["d"]:r["d"] + 1], 0.0)

        # ---- apply + store --------------------------------------------
        def apply_store(r, idx):
            a, w, d = r["a"], r["w"], r["d"]
            we = w + (w % 2)
            sc_col = scale[:, d:d + 1]
            of_col = off[:, d:d + 1]
            eng = {"v": nc.vector, "a": nc.scalar,
                   "g": nc.gpsimd}[APPLY_ENG.get(d, "v")]
            if eng is nc.scalar:
                nc.scalar.activation(ot[:, a:a + we], xt[:, a:a + we],
                                     AF.Identity, bias=of_col, scale=sc_col)
            else:
                p = a
                while p < a + we:
                    q = min(p + APPLY_CHUNK, a + we)
                    eng.tensor_scalar(
                        ot[:, p:q], xt[:, p:q], sc_col, of_col,
                        OP.mult, OP.add,
                    )
                    p = q
            nc.sync.dma_start(o_d[:, a:a + we], ot[:, a:a + we])

        # ---- emission --------------------------------------------------
        for bi, batch in enumerate(batches):
            for r in batch:
                stats(r)
            pb_eng = nc.gpsimd if bi < len(batches) - 1 else nc.vector
            phase_b(batch, pb_eng)
            for r in batch:
                apply_store(r, rngs.index(r))


_NC_CACHE = {}


def _get_nc(counts):
    key = tuple(int(c) for c in counts)
    if key not in _NC_CACHE:
        nc = _build_kernel(key)
        _split_multiwait_instructions(nc)
        _NC_CACHE[key] = nc
    return _NC_CACHE[key]


def _prep(inputs):
    x = np.asarray(inputs["x"])
    y = np.asarray(inputs["y"]).astype(np.int64)
    gamma = np.asarray(inputs["gamma"], dtype=np.float32)
    beta = np.asarray(inputs["beta"], dtype=np.float32)
    n, f = x.shape
    d = gamma.shape[0]

    counts = np.bincount(y, minlength=d).astype(np.int64)
    perm = np.argsort(y, kind="stable")
    xs = x[perm].astype(np.float16)

    rngs, S_pad = _plan(counts)
    xs_p = np.zeros((S_pad, f), dtype=np.float16)
    pos = np.empty(n, dtype=np.int64)
    o = 0
    for r in rngs:
        c = r["w"]
        xs_p[r["a"]:r["a"] + c] = xs[o:o + c]
        pos[o:o + c] = np.arange(r["a"], r["a"] + c)
        o += c

    fa = np.array([(r["cov"] / r["w"]) if r["w"] > 0 else 0.0
                   for r in rngs], dtype=np.float32)
    return x, counts, perm, pos, xs_p, gamma, beta, fa, S_pad


def _run(inputs, trace=False, **kw):
    x, counts, perm, pos, xs_p, gamma, beta, fa, S_pad = _prep(inputs)
    n, f = x.shape
    d = gamma.shape[0]

    nc = _get_nc(counts)
    in_maps = []
    for c in range(N_CORES):
        sl = slice(c * FC, (c + 1) * FC)
        aux = np.empty((FC, 3 * D + 1), dtype=np.float32)
        aux[:, 0:D] = gamma[:, sl].T
        aux[:, D:2 * D] = beta[:, sl].T
        aux[:, 2 * D] = EPS
        aux[:, 2 * D + 1:] = fa[None, :]
        in_maps.append(
            {
                "x": np.ascontiguousarray(xs_p[:, sl].T),
                "aux": aux,
            }
        )
    res = run_bass_kernel_spmd(
        nc, in_maps, core_ids=list(range(N_CORES)), trace=trace, **kw
    )
    out_s = np.empty((n, f), dtype=np.float32)
    for c in range(N_CORES):
        sl = slice(c * FC, (c + 1) * FC)
        out_s[:, sl] = res.results[c]["out"].T[pos]
    out = np.empty_like(out_s)
    out[perm] = out_s
    return out, res


def kernel(**inputs) -> np.ndarray:
    out, _ = _run(inputs, trace=False)
    return out


# revision 20
# speedup vs baseline: 1.0613x; 1.0099x over previous
"""Domain-specific BatchNorm (training mode) Trainium2 Bass kernel, v7.

Feature-sharded, collective-free: each of 8 cores owns 128 of 1024
features for ALL rows; host stable-sorts rows by domain and ships x
transposed ([128 feat, S cols], fp16) with each domain's column range
zero-padded so its bn blocks are EQUAL width (bn_aggr combines groups
with unweighted averages, so equal blocks are required for exact
variance; zeros contribute 0 to sums and are corrected exactly by
g = padded/real per domain).

Stats:
  - bn domains (all but ACT_DOMAINS): nb = ceil(w/512) equal even-width
    blocks on DVE bn_stats + one bn_aggr -> (mean, var) of the padded
    range; pb multiplies by g to recover exact stats.
  - ACT_DOMAINS (chosen where ACT is idle): Identity(scale=1/n)+accum
    and Square(scale=sqrt(1/n))+accum over the exact range.
Merge + scale/offset math on GpSimd for early batches, DVE for the last
(short tail); sqrt on ACT, reciprocal on DVE.  Apply is one tensor_scalar
(x*scale+off, 4x DVE mode) per domain, spread across GPS/ACT/DVE.
Loads per padded domain range on the sync queue (first two domains split
in half for an earlier stats start); stores on sync after each apply.
"""

import math
import os
import sys

import numpy as np

for _p in ("/opt/trn_rl_repo", "/root/.axon_site/_ro/trn_rl_repo"):
    if os.path.isdir(_p) and _p not in sys.path:
        sys.path.insert(0, _p)

import concourse.bass as bass
import concourse.tile as tile
from concourse import mybir
from concourse.bass_utils import run_bass_kernel_spmd

N_CORES = 8
N, F, D = 16384, 1024, 8
FC = F // N_CORES  # features per core (128)
EPS = 1e-5
ALIGN = 32
BNB = 512           # bn_stats max block width
NBMAX = 8
APPLY_CHUNK = 4096  # one tensor_scalar op per domain

F32 = mybir.dt.float32
F16 = mybir.dt.float16
AF = mybir.ActivationFunctionType
OP = mybir.AluOpType

ACT_DOMAINS = (0, 4)  # full stats on ACT (idle early / mid)
# apply engine per domain: v=vector(DVE), a=scalar(ACT), g=gpsimd
APPLY_ENG = {0: "g", 1: "a", 2: "g", 3: "a", 4: "g", 5: "a", 6: "v", 7: "v"}


def _split_multiwait_instructions(nc):
    """Walrus codegen encodes at most ONE sync wait per engine instruction.
    Tile may attach several; hoist all but the last into standalone
    InstEventSemaphore instructions on the same engine, placed before."""
    n = 0
    for fn in nc.m.functions:
        for block in fn.blocks:
            out = []
            for inst in block.instructions:
                si = inst.sync_info
                waits = list(si.on_wait) if si is not None else []
                if len(waits) > 1:
                    for w in waits[:-1]:
                        ev = mybir.InstEventSemaphore(
                            name=f"{inst.name}-ws{n}", ins=[], outs=[]
                        )
                        ev.engine = inst.engine
                        ev.sync_info = mybir.SyncInfo(on_wait=[w], on_update=[])
                        out.append(ev)
                        n += 1
                    inst.sync_info = mybir.SyncInfo(
                        on_wait=[waits[-1]], on_update=list(si.on_update)
                    )
                out.append(inst)
            block.instructions = out
    return n


def _plan(counts):
    """Per-domain layout: padded starts, equal bn blocks, engine split."""
    rngs = []
    a = 0
    for d, c in enumerate(counts):
        c = int(c)
        nb = bw = 0
        if c > 1 and d not in ACT_DOMAINS:
            nb = min((c + BNB - 1) // BNB, NBMAX)
            bw = 2 * ((c + 2 * nb - 1) // (2 * nb))  # even, nb*bw >= c
        cov = nb * bw  # zero-padded bn range (cov >= c, pad < 2*nb)
        rngs.append({"a": a, "w": c, "nb": nb, "bw": bw, "cov": cov, "d": d})
        span = max(cov, c)
        a += (span + ALIGN - 1) // ALIGN * ALIGN
    return rngs, max(a, ALIGN)


def _build_kernel(counts):
    rngs, S_pad = _plan(counts)
    nc = bass.Bass("TRN2", target_bir_lowering=False, debug=False,
                   num_devices=N_CORES)
    x_d = nc.dram_tensor("x", [FC, S_pad], F16, kind="ExternalInput")
    aux_d = nc.dram_tensor("aux", [FC, 3 * D + 1], F32, kind="ExternalInput")
    o_d = nc.dram_tensor("out", [FC, S_pad], F16, kind="ExternalOutput")

    with tile.TileContext(nc) as tc:
        _body(tc, counts, rngs, S_pad, x_d, aux_d, o_d)
    return nc


def _body(tc, counts, rngs, S_pad, x_d, aux_d, o_d):
    nc = tc.nc
    from contextlib import ExitStack

    live = [r for r in rngs if r["w"] > 0]
    batches = [[r for r in live if r["d"] < 3],
               [r for r in live if 3 <= r["d"] < 6],
               [r for r in live if r["d"] >= 6]]
    batches = [b for b in batches if b]

    with ExitStack() as ctx:
        big = ctx.enter_context(tc.tile_pool(name="big", bufs=1))
        small = ctx.enter_context(tc.tile_pool(name="small", bufs=1))

        xt = big.tile([FC, S_pad], F16)
        ot = big.tile([FC, S_pad], F16)
        wb_max = max((r["w"] for r in live if r["d"] in ACT_DOMAINS),
                     default=1)
        scr_a = big.tile([FC, max(wb_max, 1)], F16)   # ACT activation out

        aux = small.tile([FC, 3 * D + 1], F32)
        ga = aux[:, 0:D]
        be = aux[:, D:2 * D]
        eps_t = aux[:, 2 * D:2 * D + 1]
        fa = aux[:, 2 * D + 1:3 * D + 1]  # g = cov/w per bn domain, 0 else

        nc.gpsimd.dma_start(aux[:, :], aux_d[:, :])

        bnout = small.tile([FC, D * NBMAX * 6], F32)
        aggr = small.tile([FC, 2 * D], F32)
        meanb = small.tile([FC, D], F32)
        msqb = small.tile([FC, D], F32)
        # same-engine zero-init (consumers: aggr->DVE pb reads via GPS/DVE;
        # meanb/msqb written by ACT) keeps dep edges local
        nc.vector.memset(aggr[:, :], 0.0)
        nc.scalar.memzero(meanb[:, :])
        nc.scalar.memzero(msqb[:, :])

        m2a = small.tile([FC, D], F32)
        ex2 = small.tile([FC, D], F32)
        mean = small.tile([FC, D], F32)
        mt2 = small.tile([FC, D], F32)
        var = small.tile([FC, D], F32)
        sd = small.tile([FC, D], F32)
        inv = small.tile([FC, D], F32)
        scale = small.tile([FC, D], F32)
        ms = small.tile([FC, D], F32)
        off = small.tile([FC, D], F32)

        # ---- loads: one padded range per domain; first bn domain first
        # (its 512-blocks can start on a half chunk) then the ACT domains
        first_bn = next((r["d"] for r in live if r["nb"] > 0), None)
        order = sorted(range(len(rngs)),
                       key=lambda i: (0 if i == first_bn else 1, i))
        for i in order:
            r = rngs[i]
            a, w = r["a"], r["w"]
            span = max(r["cov"], w + (w % 2))
            if w <= 0:
                continue
            if i == first_bn and r["nb"] >= 2:
                h = (r["nb"] // 2) * r["bw"]
                nc.sync.dma_start(xt[:, a:a + h], x_d[:, a:a + h])
                nc.sync.dma_start(xt[:, a + h:a + span], x_d[:, a + h:a + span])
            else:
                nc.sync.dma_start(xt[:, a:a + span], x_d[:, a:a + span])

        # ---- stats per domain ----------------------------------------
        def stats(r):
            a, w, nb, bw, d = r["a"], r["w"], r["nb"], r["bw"], r["d"]
            if w <= 1:
                return
            if nb > 0:
                for k in range(nb):
                    nc.vector.bn_stats(
                        bnout[:, (d * NBMAX + k) * 6:(d * NBMAX + k + 1) * 6],
                        xt[:, a + k * bw:a + (k + 1) * bw],
                    )
                nc.vector.bn_aggr(
                    aggr[:, 2 * d:2 * d + 2],
                    bnout[:, d * NBMAX * 6:(d * NBMAX) * 6 + nb * 6],
                )
            else:
                rc = 1.0 / w
                xin = xt[:, a:a + w]
                nc.scalar.activation(
                    scr_a[:, 0:w], xin, AF.Identity, scale=rc,
                    accum_out=meanb[:, d:d + 1],
                )
                nc.scalar.activation(
                    scr_a[:, 0:w], xin, AF.Square, scale=math.sqrt(rc),
                    accum_out=msqb[:, d:d + 1],
                )

        # ---- merge + scale/offset math for a batch --------------------
        def phase_b(batch, e):
            lo = min(r["d"] for r in batch)
            hi = max(r["d"] for r in batch) + 1
            cs = slice(lo, hi)
            ma = aggr[:, 2 * lo:2 * hi:2]
            va = aggr[:, 2 * lo + 1:2 * hi:2]
            e.tensor_tensor(m2a[:, cs], ma, ma, OP.mult)
            e.tensor_tensor(ex2[:, cs], va, m2a[:, cs], OP.add)
            e.tensor_tensor(ex2[:, cs], ex2[:, cs], fa[:, cs], OP.mult)
            e.tensor_tensor(ex2[:, cs], ex2[:, cs], msqb[:, cs], OP.add)
            e.tensor_tensor(mean[:, cs], ma, fa[:, cs], OP.mult)
            e.tensor_tensor(mean[:, cs], mean[:, cs], meanb[:, cs], OP.add)
            e.tensor_tensor(mt2[:, cs], mean[:, cs], mean[:, cs], OP.mult)
            e.tensor_tensor(var[:, cs], ex2[:, cs], mt2[:, cs], OP.subtract)
            nc.scalar.activation(sd[:, cs], var[:, cs], AF.Sqrt,
                                 bias=eps_t[:, 0:1])
            nc.vector.reciprocal(inv[:, cs], sd[:, cs])
            e.tensor_tensor(scale[:, cs], inv[:, cs], ga[:, cs], OP.mult)
            e.tensor_tensor(ms[:, cs], mean[:, cs], scale[:, cs], OP.mult)
            e.tensor_tensor(off[:, cs], be[:, cs], ms[:, cs], OP.subtract)
            for r in batch:
                if r["w"] == 1:  # passthrough: out = x
                    nc.vector.memset(scale[:, r["d"]:r["d"] + 1], 1.0)
                    nc.vector.memset(off[:, r["d"]:r["d"] + 1], 0.0)

        # ---- apply + store --------------------------------------------
        def apply_store(r):
            a, w, d = r["a"], r["w"], r["d"]
            we = w + (w % 2)
            sc_col = scale[:, d:d + 1]
            of_col = off[:, d:d + 1]
            eng = {"v": nc.vector, "a": nc.scalar,
                   "g": nc.gpsimd}[APPLY_ENG.get(d, "v")]
            if eng is nc.scalar:
                nc.scalar.activation(ot[:, a:a + we], xt[:, a:a + we],
                                     AF.Identity, bias=of_col, scale=sc_col)
            else:
                p = a
                while p < a + we:
                    q = min(p + APPLY_CHUNK, a + we)
                    eng.tensor_scalar(
                        ot[:, p:q], xt[:, p:q], sc_col, of_col,
                        OP.mult, OP.add,
                    )
                    p = q
            nc.sync.dma_start(o_d[:, a:a + we], ot[:, a:a + we])

        # ---- emission --------------------------------------------------
        for bi, batch in enumerate(batches):
            for r in batch:
                stats(r)
            pb_eng = nc.gpsimd if bi < len(batches) - 1 else nc.vector
            phase_b(batch, pb_eng)
            for r in batch:
                apply_store(r)


_NC_CACHE = {}


def _get_nc(counts):
    key = tuple(int(c) for c in counts)
    if key not in _NC_CACHE:
        nc = _build_kernel(key)
        _split_multiwait_instructions(nc)
        _NC_CACHE[key] = nc
    return _NC_CACHE[key]


def _prep(inputs):
    x = np.asarray(inputs["x"])
    y = np.asarray(inputs["y"]).astype(np.int64)
    gamma = np.asarray(inputs["gamma"], dtype=np.float32)
    beta = np.asarray(inputs["beta"], dtype=np.float32)
    n, f = x.shape
    d = gamma.shape[0]

    counts = np.bincount(y, minlength=d).astype(np.int64)
    perm = np.argsort(y, kind="stable")
    xs = x[perm].astype(np.float16)

    rngs, S_pad = _plan(counts)
    xs_p = np.zeros((S_pad, f), dtype=np.float16)
    pos = np.empty(n, dtype=np.int64)
    o = 0
    for r in rngs:
        c = r["w"]
        xs_p[r["a"]:r["a"] + c] = xs[o:o + c]
        pos[o:o + c] = np.arange(r["a"], r["a"] + c)
        o += c

    fa = np.array([(r["cov"] / r["w"]) if (r["w"] > 0 and r["nb"] > 0)
                   else 0.0 for r in rngs], dtype=np.float32)
    return x, counts, perm, pos, xs_p, gamma, beta, fa, S_pad


def _run(inputs, trace=False, **kw):
    x, counts, perm, pos, xs_p, gamma, beta, fa, S_pad = _prep(inputs)
    n, f = x.shape
    d = gamma.shape[0]

    nc = _get_nc(counts)
    in_maps = []
    for c in range(N_CORES):
        sl = slice(c * FC, (c + 1) * FC)
        aux = np.empty((FC, 3 * D + 1), dtype=np.float32)
        aux[:, 0:D] = gamma[:, sl].T
        aux[:, D:2 * D] = beta[:, sl].T
        aux[:, 2 * D] = EPS
        aux[:, 2 * D + 1:] = fa[None, :]
        in_maps.append(
            {
                "x": np.ascontiguousarray(xs_p[:, sl].T),
                "aux": aux,
            }
        )
    res = run_bass_kernel_spmd(
        nc, in_maps, core_ids=list(range(N_CORES)), trace=trace, **kw
    )
    out_s = np.empty((n, f), dtype=np.float32)
    for c in range(N_CORES):
        sl = slice(c * FC, (c + 1) * FC)
        out_s[:, sl] = res.results[c]["out"].T[pos]
    out = np.empty_like(out_s)
    out[perm] = out_s
    return out, res


def kernel(**inputs) -> np.ndarray:
    out, _ = _run(inputs, trace=False)
    return out
